# revision 2
# baseline (speedup 1.0000x reference)
"""ApproxEMD Trainium2 kernel v3 — two-batch interleaved pipeline.

Differences vs kernel2 (single-batch):
  * P is built on the HOST (fp32 -> bf16) and streamed from DRAM per
    exp — no device P build, no resident P (frees ~64KB/partition).
  * The per-core 2 batches are phase-interleaved: batch B's s-pass and
    exps run while batch A's r-pass occupies the DVE, and vice versa.
    Every engine gets back-to-back work each round.
  * alpha broadcast lands in a 1-bank PSUM mini-ring ([128,512] chunks)
    copied straight to absb, so both batches' s-pass PSUM fits.
  * u-quad updates + lhsT refresh run on the Pool engine (one unit of
    slack before the owning batch's next s-pass needs them).
"""

import numpy as np

import concourse.bass as bass
import concourse.mybir as mybir
import concourse.tile as tile
from concourse import bacc
from concourse.bass_utils import run_bass_kernel_spmd

FP32 = mybir.dt.float32
BF16 = mybir.dt.bfloat16
AF = mybir.ActivationFunctionType
OP = mybir.AluOpType

B, N, D = 16, 2048, 3
NCORES = 8
BPC = B // NCORES
NT = N // 128
M = N
MC = M // 512
EPS = 1e-9
EXP_FACTORS = [-(4.0 ** i) for i in range(7, -2, -1)] + [0.0]
NIT = len(EXP_FACTORS) - 1
E_BUFS = 32
PD_BUFS = 5


def build_program(n_batches=BPC):
    nc = bacc.Bacc("TRN2", target_bir_lowering=False, debug=False,
                   num_devices=NCORES)
    pbf_d = nc.dram_tensor("pbf", [BPC, NT, 128, M], BF16,
                           kind="ExternalInput").ap()
    pcols_d = nc.dram_tensor("pcols", [BPC, 128, 5, NT], FP32,
                             kind="ExternalInput").ap()
    lrows_d = nc.dram_tensor("lrows", [BPC, 16, 5, 128], FP32,
                             kind="ExternalInput").ap()
    sacc_d = nc.dram_tensor("sacc", [16, 128], FP32, kind="ExternalOutput").ap()
    wfin_d = nc.dram_tensor("wfin", [BPC, 20, 1], FP32, kind="ExternalOutput").ap()
    cfin_d = nc.dram_tensor("cfin", [BPC, 16, 128], FP32, kind="ExternalOutput").ap()

    with tile.TileContext(nc) as tc:
        with (
            tc.tile_pool(name="pE", bufs=E_BUFS) as pE,
            tc.tile_pool(name="pPd", bufs=PD_BUFS) as pPd,
            tc.tile_pool(name="pH", bufs=2) as pH,
            tc.tile_pool(name="pAB", bufs=2) as pAB,
            tc.tile_pool(name="pSR", bufs=1) as pSR,
            tc.tile_pool(name="pSm", bufs=1) as pSm,
            tc.tile_pool(name="pPS", bufs=1, space=bass.MemorySpace.PSUM) as pPS,
            tc.tile_pool(name="pPSb", bufs=2, space=bass.MemorySpace.PSUM) as pPSb,
        ):
            sacc16 = pSm.tile([16, 128], FP32, tag="sacc16")
            onesrow = pSm.tile([1, 128], BF16, tag="onesrow")
            onescol = pSm.tile([128, 1], BF16, tag="onescol")
            pcolsF = pSm.tile([128, 5, NT], FP32, tag="pcolsF")

            nc.vector.memset(sacc16[:], 0.0)
            nc.vector.memset(onesrow[:], 1.0)
            nc.vector.memset(onescol[:], 1.0)

            # ---- per-batch state ----
            St = []
            for b in range(n_batches):
                s = {}
                s["ub6q"] = [pSm.tile([128, 6, 4], BF16, tag=f"ub6q{b}_{q}",
                                      name=f"ub6q{b}_{q}") for q in range(4)]
                s["pcols6"] = pSm.tile([128, 5, NT], BF16, tag=f"pcols6_{b}",
                                       name=f"pcols6_{b}")
                s["lrow"] = pSm.tile([16, 5, 128], FP32, tag=f"lrow_{b}",
                                     name=f"lrow_{b}")
                s["ucol"] = pSm.tile([128, NT], FP32, tag=f"ucol_{b}",
                                     name=f"ucol_{b}")
                s["rcol"] = pSm.tile([128, NT], FP32, tag=f"rcol_{b}",
                                     name=f"rcol_{b}")
                s["tmpU"] = pSm.tile([128, NT], FP32, tag=f"tmpU_{b}",
                                     name=f"tmpU_{b}")
                s["v6"] = pSm.tile([16, 6, 128], FP32, tag=f"v6_{b}",
                                   name=f"v6_{b}")
                for nm in ("c16", "t16", "bw16", "al16", "tmpA", "tmpB",
                           "tmpS"):
                    s[nm] = pSm.tile([16, 128], FP32, tag=f"{nm}_{b}",
                                     name=f"{nm}_{b}")
                s["t5"] = pSm.tile([16, 5, 128], FP32, tag=f"t5_{b}",
                                   name=f"t5_{b}")
                s["a16b"] = pSm.tile([16, 128], BF16, tag=f"a16b_{b}",
                                     name=f"a16b_{b}")
                s["arowb"] = pSm.tile([1, M], BF16, tag=f"arowb_{b}",
                                      name=f"arowb_{b}")
                St.append(s)

            # ---- init + iteration-0 E for both batches ----
            Et = [None] * n_batches
            for b in range(n_batches):
                s = St[b]
                nc.sync.dma_start(pcolsF[:], pcols_d[b])
                nc.sync.dma_start(s["lrow"][:], lrows_d[b])
                nc.vector.tensor_copy(s["pcols6"][:], pcolsF[:])
                for q in range(4):
                    nc.vector.memset(s["ub6q"][q][:, 0:1, :], 1.0)
                    nc.vector.tensor_copy(s["ub6q"][q][:, 1:6, :],
                                          s["pcols6"][:, :, 4 * q:4 * q + 4])
                nc.vector.memset(s["ucol"][:], 1.0)
                nc.vector.memset(s["c16"][:], 1.0)

                Et[b] = []
                for i in range(NT):
                    Pd = pPd.tile([128, M], BF16, tag="Pd",
                                  name=f"Pd{i}_{b}_0")
                    nc.sync.dma_start(Pd[:], pbf_d[b, i])
                    E = pE.tile([128, M], BF16, tag="E", name=f"E{i}_{b}_0")
                    nc.scalar.activation(E[:], Pd[:], AF.Exp,
                                         scale=float(EXP_FACTORS[0]))
                    Et[b].append(E)

            # ---- interleaved auction iterations ----
            for t in range(NIT):
                for b in range(n_batches):
                    s = St[b]
                    # s-pass
                    ps6 = pPS.tile([128, M], FP32, tag="ps",
                                   name=f"ps6_{b}_{t}")
                    for i in range(NT):
                        for c in range(MC):
                            nc.tensor.matmul(
                                ps6[0:6, 512 * c:512 * (c + 1)],
                                s["ub6q"][i // 4][:, :, i % 4:i % 4 + 1],
                                Et[b][i][:, 512 * c:512 * (c + 1)],
                                start=(i == 0), stop=(i == NT - 1),
                            )
                    srow6 = pSR.tile([6, M], FP32, tag="srow6",
                                     name=f"srow6_{b}_{t}")
                    nc.vector.tensor_copy(srow6[:], ps6[0:6, :])
                    for k in range(6):
                        eng = nc.sync if k % 2 == 0 else nc.scalar
                        eng.dma_start(s["v6"][:, k, :], srow6[k:k + 1, :])

                    # per-m math
                    s0_16 = s["v6"][:, 0, :]
                    s1_16 = s["v6"][:, 5, :]
                    nc.vector.tensor_tensor(s["tmpA"][:], s["c16"][:], s0_16,
                                            OP.mult)
                    nc.vector.tensor_scalar_add(s["tmpA"][:], s["tmpA"][:], EPS)
                    nc.vector.reciprocal(s["tmpA"][:], s["tmpA"][:])
                    nc.vector.tensor_tensor(s["tmpB"][:], s["c16"][:], s1_16,
                                            OP.mult)
                    nc.vector.tensor_tensor(s["tmpB"][:], s["tmpB"][:],
                                            s["tmpA"][:], OP.mult)
                    nc.vector.tensor_scalar_add(s["bw16"][:], s["tmpB"][:], EPS)
                    nc.vector.reciprocal(s["bw16"][:], s["bw16"][:])
                    nc.vector.tensor_tensor(s["bw16"][:], s["bw16"][:],
                                            s["c16"][:], OP.mult)
                    nc.vector.tensor_scalar_min(s["bw16"][:], s["bw16"][:], 1.0)
                    nc.vector.tensor_tensor(s["al16"][:], s["bw16"][:],
                                            s["tmpA"][:], OP.mult)
                    nc.vector.tensor_tensor(s["al16"][:], s["al16"][:],
                                            s["c16"][:], OP.mult)
                    nc.vector.tensor_tensor(s["tmpB"][:], s["tmpB"][:],
                                            s["bw16"][:], OP.mult)
                    nc.vector.tensor_tensor(s["c16"][:], s["c16"][:],
                                            s["tmpB"][:], OP.subtract)
                    nc.vector.tensor_scalar_max(s["c16"][:], s["c16"][:], 0.0)
                    nc.vector.tensor_copy(s["a16b"][:], s["al16"][:])

                    # t assembly + S accumulation (Pool)
                    nc.gpsimd.tensor_tensor(s["t5"][:], s["lrow"][:],
                                            s["v6"][:, 1:6, :], OP.mult)
                    nc.gpsimd.tensor_tensor(s["t16"][:], s["t5"][:, 0, :],
                                            s["t5"][:, 1, :], OP.add)
                    nc.gpsimd.tensor_tensor(s["t16"][:], s["t16"][:],
                                            s["t5"][:, 2, :], OP.add)
                    nc.gpsimd.tensor_tensor(s["t16"][:], s["t16"][:],
                                            s["t5"][:, 3, :], OP.add)
                    nc.gpsimd.tensor_tensor(s["t16"][:], s["t16"][:],
                                            s["t5"][:, 4, :], OP.add)
                    nc.gpsimd.tensor_tensor(s["tmpS"][:], s["al16"][:],
                                            s["t16"][:], OP.mult)
                    nc.gpsimd.tensor_tensor(sacc16[:], sacc16[:], s["tmpS"][:],
                                            OP.add)

                    # alpha broadcast via 1-bank psum chunks
                    nc.sync.dma_start(s["arowb"][:], s["a16b"][:])
                    absb = pAB.tile([128, M], BF16, tag="absb",
                                    name=f"absb_{b}_{t}")
                    for c in range(MC):
                        abm = pPSb.tile([128, 512], FP32, tag="abm",
                                        name=f"abm_{b}_{t}_{c}")
                        nc.tensor.matmul(
                            abm[:],
                            onesrow[:],
                            s["arowb"][0:1, 512 * c:512 * (c + 1)],
                            start=True, stop=True,
                        )
                        nc.scalar.copy(absb[:, 512 * c:512 * (c + 1)], abm[:])

                    # r-pass (DVE) + u updates (Pool)
                    for i in range(NT):
                        H = pH.tile([128, M], BF16, tag="H",
                                    name=f"H_{b}_{t}_{i}")
                        nc.vector.scalar_tensor_tensor(
                            H[:], Et[b][i][:], 1.0, absb[:],
                            OP.mult, OP.mult,
                            accum_out=s["rcol"][:, i:i + 1])
                        if i % 4 == 3:
                            q = i - 3
                            nc.gpsimd.tensor_tensor(s["tmpU"][:, q:q + 4],
                                                    s["ucol"][:, q:q + 4],
                                                    s["rcol"][:, q:q + 4],
                                                    OP.mult)
                            nc.gpsimd.tensor_tensor(s["ucol"][:, q:q + 4],
                                                    s["ucol"][:, q:q + 4],
                                                    s["tmpU"][:, q:q + 4],
                                                    OP.subtract)
                            nc.gpsimd.tensor_scalar_max(s["ucol"][:, q:q + 4],
                                                        s["ucol"][:, q:q + 4],
                                                        0.0)
                            for k in range(5):
                                nc.gpsimd.tensor_tensor(
                                    s["ub6q"][i // 4][:, 1 + k, :],
                                    s["pcols6"][:, k, q:q + 4],
                                    s["ucol"][:, q:q + 4], OP.mult)

                    # next iteration's E (+ P prefetch), after the r-loop
                    if t + 1 < NIT:
                        Enx = []
                        for i in range(NT):
                            Pd = pPd.tile([128, M], BF16, tag="Pd",
                                          name=f"Pd{i}_{b}_{t + 1}")
                            nc.sync.dma_start(Pd[:], pbf_d[b, i])
                            E = pE.tile([128, M], BF16, tag="E",
                                        name=f"E{i}_{b}_{t + 1}")
                            nc.scalar.activation(E[:], Pd[:], AF.Exp,
                                                 scale=float(EXP_FACTORS[t + 1]))
                            Enx.append(E)
                        Et[b] = Enx

            # ---- final: w_k = sum_n u*paug_k, c out ----
            for b in range(n_batches):
                s = St[b]
                wps = pPS.tile([128, M], FP32, tag="ps", name=f"wps_{b}")
                for q in range(4):
                    nc.tensor.matmul(wps[0:20, 0:1],
                                     s["ub6q"][q][:, 1:6, :], onescol[:],
                                     start=(q == 0), stop=(q == 3))
                w20 = pSm.tile([20, 1], FP32, tag=f"w20_{b}", name=f"w20_{b}")
                nc.vector.tensor_copy(w20[:], wps[0:20, 0:1])
                nc.sync.dma_start(wfin_d[b], w20[:])
                nc.sync.dma_start(cfin_d[b], s["c16"][:])

            nc.sync.dma_start(sacc_d[:], sacc16[:])

    nc.compile()
    return nc


_CACHED = None


def _get_program():
    global _CACHED
    if _CACHED is None:
        _CACHED = build_program()
    return _CACHED


def _augment(preds, labels):
    pn = np.sum(preds.astype(np.float64) ** 2, axis=-1)
    ln = np.sum(labels.astype(np.float64) ** 2, axis=-1)
    Bn = preds.shape[0]
    paug = np.empty((Bn, 5, N), np.float32)
    laug = np.empty((Bn, 5, M), np.float32)
    paug[:, 0:3, :] = -2.0 * np.transpose(preds, (0, 2, 1))
    paug[:, 3, :] = pn
    paug[:, 4, :] = 1.0
    laug[:, 0:3, :] = np.transpose(labels, (0, 2, 1))
    laug[:, 3, :] = 1.0
    laug[:, 4, :] = ln
    pcols = np.ascontiguousarray(
        paug.reshape(Bn, 5, NT, 128).transpose(0, 3, 1, 2))
    lrows = np.ascontiguousarray(
        laug.reshape(Bn, 5, 16, 128).transpose(0, 2, 1, 3))
    return paug, laug, pcols, lrows


def _build_pbf(paug, laug):
    """Host P build: fp32 matmul -> bf16 row tiles [B, NT, 128, M]."""
    import ml_dtypes
    Bn = paug.shape[0]
    P = np.einsum("bkn,bkm->bnm", paug, laug).astype(np.float32)
    P = P.astype(ml_dtypes.bfloat16)
    return np.ascontiguousarray(P.reshape(Bn, NT, 128, M))


def _host_final_iteration(w, c, laug_b):
    w = w.astype(np.float64)
    c = c.astype(np.float64)
    tvec = (laug_b.astype(np.float64) * w[:, None]).sum(0)
    su = w[4]
    denom1 = c * float(N) + EPS
    d2 = c * su / denom1
    bid_wt = np.minimum(c / (d2 + EPS), 1.0)
    alpha = c * bid_wt / denom1
    return float(np.sum(alpha * tvec))


def kernel(preds, labels):
    preds = np.asarray(preds, dtype=np.float32)
    labels = np.asarray(labels, dtype=np.float32)
    paug, laug, pcols, lrows = _augment(preds, labels)
    pbf = _build_pbf(paug, laug)

    in_maps = []
    for core in range(NCORES):
        sl = slice(core * BPC, (core + 1) * BPC)
        in_maps.append({
            "pbf": np.ascontiguousarray(pbf[sl]),
            "pcols": np.ascontiguousarray(pcols[sl]),
            "lrows": np.ascontiguousarray(lrows[sl]),
        })

    nc = _get_program()
    res = run_bass_kernel_spmd(nc, in_maps, core_ids=list(range(NCORES)))

    total = 0.0
    for core in range(NCORES):
        out = res.results[core]
        total += float(np.sum(out["sacc"].astype(np.float64)))
        for b in range(BPC):
            w = out["wfin"][b].reshape(5, 4).sum(axis=1)
            total += _host_final_iteration(
                w,
                out["cfin"][b].reshape(-1),
                laug[core * BPC + b],
            )
    return np.float32(total)
